# revision 16
# baseline (speedup 1.0000x reference)
"""AFT (attention-free transformer) block on 8 TRN2 NeuronCores — v2.

Reference computation (T=2048, B=4, D=1024):
    qkv = data @ W_qkv + b_qkv ; q,k,v = split(qkv)
    num = exp(pb - max_pb) @ (exp(k - max_k) * v)    (contraction over key pos j)
    den = exp(pb - max_pb) @ exp(k - max_k)
    out = (sigmoid(q) * num / den) @ W_out + b_out
The max shifts cancel exactly in num/den and value ranges are tiny, so the
kernel drops them.

Sharding: sequence-parallel over the query axis i; core c owns i in
[c*256,(c+1)*256). Each core computes q/k/v for its own rows, all-gathers
exp(k) and exp(k)*v (bf16, two pipelined chunks), then computes its num/den
rows and the output projection.

v2 structure (vs v1): every matmul loop reuses one stationary (lhsT) load
for 2-4 N=512 moving passes (the compile config runs with ldw-opt off, so
LDWEIGHTS serialize with matmuls); num/den keep exp(pbT) stationary (shared
across num, den and both d-chunks); y comes out token-major and is
PE-transposed (64x 128x128) for the output projection; sigmoid(q) is bounced
through DRAM to get the batch-separated layout; the AllGather is split into
two chunks so it pipelines with qkv production and num/den consumption.
"""

import numpy as np
import ml_dtypes

from concourse import bacc, bass, mybir, tile
from concourse.bass_utils import run_bass_kernel_spmd
from concourse.masks import make_identity

BF16 = mybir.dt.bfloat16
F32 = mybir.dt.float32
AF = mybir.ActivationFunctionType

N_CORES = 8
T, B, D = 2048, 4, 1024
TL = T // N_CORES          # 256 local query rows
TOK = TL * B               # 1024 local tokens
KT = D // 128              # 8 contraction tiles for d
JT = T // 128              # 16 j tiles

_cache = {}


def build(with_qkv_bias: bool, with_out_bias: bool):
    nc = bacc.Bacc(None, target_bir_lowering=False)

    dataT_d = nc.dram_tensor("dataT", [D, TOK], BF16, kind="ExternalInput")
    wqkv_d = nc.dram_tensor("wqkv", [D, 3 * D], BF16, kind="ExternalInput")
    pbT_d = nc.dram_tensor("pbT", [T, TL], BF16, kind="ExternalInput")
    wout_d = nc.dram_tensor("wout", [D, D], BF16, kind="ExternalInput")
    out_d = nc.dram_tensor("out", [TOK, D], F32, kind="ExternalOutput")
    if with_qkv_bias:
        bqkv_d = nc.dram_tensor("bqkv", [1, 3 * D], BF16, kind="ExternalInput")
    if with_out_bias:
        bout_d = nc.dram_tensor("bout", [1, D], BF16, kind="ExternalInput")

    with tile.TileContext(nc) as tc:
        with (
            tc.tile_pool(name="persist", bufs=1) as pp,
            tc.tile_pool(name="psum_mm", bufs=8, space="PSUM") as psmm,
            tc.tile_pool(name="dram", bufs=1, space="DRAM") as dram,
        ):
            pstr = psmm
            # ---- persistent SBUF tensors ----
            ident = pp.tile([128, 128], BF16, name="ident", tag="ident")
            make_identity(nc, ident[:])
            wout = [pp.tile([128, D], BF16, name=f"wout{k}", tag=f"wout{k}")
                    for k in range(KT)]
            pbe = [pp.tile([128, TL], BF16, name=f"pbe{t}", tag=f"pbe{t}")
                   for t in range(JT)]
            need_bias_ones = with_qkv_bias or with_out_bias
            if need_bias_ones:
                ones1 = pp.tile([1, 128], BF16, name="ones1", tag="ones1")
                nc.gpsimd.memset(ones1[:], 1.0)
            if with_qkv_bias:
                bqkv = pp.tile([1, 3 * D], BF16, name="bqkv", tag="bqkv")
                nc.sync.dma_start(bqkv[:], bqkv_d[:])
            if with_out_bias:
                bout = pp.tile([1, D], BF16, name="bout", tag="bout")
                nc.sync.dma_start(bout[:], bout_d[:])

            # collective bounce buffers: two token-half chunks of [ek | ekv]
            cc_in = [dram.tile([TOK, D], BF16, name=f"cc_in{x}") for x in range(2)]
            cc_out = [dram.tile([N_CORES * TOK, D], BF16, name=f"cc_out{x}",
                                addr_space="Shared") for x in range(2)]
            sigq_d = dram.tile([TOK, D], BF16, name="sigq_d")

            # ---- phase A: fused qkv projection ----
            with tc.tile_pool(name="phaseA", bufs=1) as pa:
                dataT = [pa.tile([128, TOK], BF16, name=f"dataT{k}",
                                 tag=f"dataT{k}") for k in range(KT)]
                wqkv = [pa.tile([128, 3 * D], BF16, name=f"wqkv{k}",
                                tag=f"wqkv{k}") for k in range(KT)]
                # interleave so matmuls can start after the first k pair lands
                for k in range(KT):
                    nc.sync.dma_start(dataT[k][:], dataT_d[k * 128:(k + 1) * 128, :])
                    nc.sync.dma_start(wqkv[k][:], wqkv_d[k * 128:(k + 1) * 128, :])
                for k in range(KT):
                    nc.sync.dma_start(wout[k][:], wout_d[k * 128:(k + 1) * 128, :])

                # pass 1: k and v chunks only (n 2..5) so the all-gather of
                # exp(k)/exp(k)*v starts as early as possible
                for m in range(KT):  # token tile
                    ek = pa.tile([128, D], BF16, name=f"ek{m}", tag="ek", bufs=2)
                    vv = pa.tile([128, D], BF16, name=f"vv{m}", tag="vv", bufs=2)
                    ekv = pa.tile([128, D], BF16, name=f"ekv{m}", tag="ekv", bufs=2)
                    ps = [psmm.tile([128, 512], F32, name=f"ps{m}_{i}",
                                    tag="ps") for i in range(4)]
                    for k in range(KT):
                        for i in range(4):
                            n = 2 + i
                            nc.tensor.matmul(
                                ps[i][:], dataT[k][:, m * 128:(m + 1) * 128],
                                wqkv[k][:, n * 512:(n + 1) * 512],
                                start=(k == 0),
                                stop=(k == KT - 1 and not with_qkv_bias),
                            )
                    if with_qkv_bias:
                        for i in range(4):
                            n = 2 + i
                            nc.tensor.matmul(
                                ps[i][:], ones1[:],
                                bqkv[:, n * 512:(n + 1) * 512],
                                start=False, stop=True,
                            )
                    for i in range(2):
                        nc.scalar.activation(
                            ek[:, i * 512:(i + 1) * 512], ps[i][:], AF.Exp)
                        nc.vector.tensor_copy(
                            vv[:, i * 512:(i + 1) * 512], ps[2 + i][:])
                    nc.vector.tensor_mul(ekv[:], ek[:], vv[:])
                    # chunk x = m//4 holds token rows [x*512,(x+1)*512):
                    # layout [ek half | ekv half]
                    x, mm = m // 4, m % 4
                    nc.sync.dma_start(
                        cc_in[x][mm * 128:(mm + 1) * 128, :], ek[:])
                    nc.sync.dma_start(
                        cc_in[x][512 + mm * 128:512 + (mm + 1) * 128, :], ekv[:])
                    if m in (3, 7):
                        nc.gpsimd.collective_compute(
                            "AllGather", mybir.AluOpType.bypass,
                            replica_groups=[list(range(N_CORES))],
                            ins=[cc_in[m // 4][:].opt()],
                            outs=[cc_out[m // 4][:].opt()],
                        )

                # pass 2: q chunks (n 0..1) + sigmoid — overlaps the collectives
                for m in range(KT):
                    sq = pa.tile([128, D], BF16, name=f"sq{m}", tag="sq", bufs=2)
                    ps = [psmm.tile([128, 512], F32, name=f"psq{m}_{i}",
                                    tag="ps") for i in range(2)]
                    for k in range(KT):
                        for i in range(2):
                            nc.tensor.matmul(
                                ps[i][:], dataT[k][:, m * 128:(m + 1) * 128],
                                wqkv[k][:, i * 512:(i + 1) * 512],
                                start=(k == 0),
                                stop=(k == KT - 1 and not with_qkv_bias),
                            )
                    if with_qkv_bias:
                        for i in range(2):
                            nc.tensor.matmul(
                                ps[i][:], ones1[:], bqkv[:, i * 512:(i + 1) * 512],
                                start=False, stop=True,
                            )
                    for i in range(2):
                        nc.scalar.activation(
                            sq[:, i * 512:(i + 1) * 512], ps[i][:], AF.Sigmoid)
                    nc.sync.dma_start(sigq_d[m * 128:(m + 1) * 128, :], sq[:])

                # exp(pbT) — also overlaps the collectives
                for t in range(JT):
                    praw = pa.tile([128, TL], BF16, name=f"praw{t}", tag="praw",
                                   bufs=2)
                    nc.sync.dma_start(praw[:], pbT_d[t * 128:(t + 1) * 128, :])
                    nc.scalar.activation(pbe[t][:], praw[:], AF.Exp)

            # ---- phase B: num/den + y + output projection ----
            # chunk x gathered rows: r*1024 + h*512 + p*4 + b  (p = local j in tile)
            ccv = [cc_out[x][:].rearrange("(r h p b) d -> r b p h d",
                                          r=N_CORES, h=2, p=128, b=B)
                   for x in range(2)]
            sqv = sigq_d[:].rearrange("(m p b) d -> m b p d", m=2, p=128, b=B)
            out_v = out_d[:].rearrange("(m p b) d -> m b p d", m=2, p=128, b=B)

            with tc.tile_pool(name="phaseB", bufs=1) as pbp:
                for b in range(B):
                    # gathered tiles: ekg[x][r] = [128, (h d)] ; j tile t = 2r+x
                    ekg = [[None] * N_CORES for _ in range(2)]
                    for x in range(2):
                        for r in range(N_CORES):
                            g = pbp.tile([128, 2048], BF16, name=f"ekg{b}_{x}_{r}",
                                         tag="ekg", bufs=20)
                            gv = g[:].rearrange("p (h d) -> p h d", h=2)
                            nc.sync.dma_start(gv, ccv[x][r, b])
                            ekg[x][r] = g
                    sqb = [pbp.tile([128, D], BF16, name=f"sqb{b}_{m2}",
                                    tag="sqb", bufs=4) for m2 in range(2)]
                    for m2 in range(2):
                        nc.sync.dma_start(sqb[m2][:], sqv[m2, b])

                    yT = [pbp.tile([128, TL], BF16, name=f"yT{b}_{k}",
                                   tag=f"yT{k}", bufs=2) for k in range(KT)]
                    for m2 in range(2):  # query i tile
                        pn = [psmm.tile([128, 512], F32, name=f"pn{b}_{m2}_{i}",
                                        tag="ps") for i in range(2)]
                        pd = [psmm.tile([128, 512], F32, name=f"pd{b}_{m2}_{i}",
                                        tag="ps") for i in range(2)]
                        # accumulate chunk-0 j tiles first (even t), then chunk 1
                        order = [(x, r) for x in range(2) for r in range(N_CORES)]
                        for idx, (x, r) in enumerate(order):
                            t = 2 * r + x
                            first, last = idx == 0, idx == len(order) - 1
                            for i in range(2):  # d chunk
                                nc.tensor.matmul(
                                    pn[i][:], pbe[t][:, m2 * 128:(m2 + 1) * 128],
                                    ekg[x][r][:, D + i * 512:D + (i + 1) * 512],
                                    start=first, stop=last)
                                nc.tensor.matmul(
                                    pd[i][:], pbe[t][:, m2 * 128:(m2 + 1) * 128],
                                    ekg[x][r][:, i * 512:(i + 1) * 512],
                                    start=first, stop=last)
                        y = pbp.tile([128, D], BF16, name=f"y{b}_{m2}", tag="y",
                                     bufs=3)
                        for i in range(2):
                            rec = pbp.tile([128, 512], F32, name=f"rec{b}{m2}{i}",
                                           tag="rec", bufs=3)
                            tmp = pbp.tile([128, 512], F32, name=f"tmp{b}{m2}{i}",
                                           tag="tmp", bufs=3)
                            nc.vector.reciprocal_approx_fast(rec[:], pd[i][:])
                            nc.vector.tensor_mul(tmp[:], pn[i][:], rec[:])
                            nc.vector.tensor_mul(
                                y[:, i * 512:(i + 1) * 512], tmp[:],
                                sqb[m2][:, i * 512:(i + 1) * 512])
                        # transpose y [i, d] -> yT [d, i] via PE, 128x128 blocks
                        for k in range(KT):
                            pt = pstr.tile([128, 128], BF16, name=f"pt{b}{m2}{k}",
                                           tag="ps")
                            nc.tensor.transpose(
                                pt[:], y[:, k * 128:(k + 1) * 128], ident[:])
                            nc.vector.tensor_copy(
                                yT[k][:, m2 * 128:(m2 + 1) * 128], pt[:])

                    po = [psmm.tile([128, 512], F32, name=f"po{b}_{m2}_{n}",
                                    tag="ps")
                          for m2 in range(2) for n in range(2)]
                    for k in range(KT):
                        for m2 in range(2):
                            for n in range(2):
                                nc.tensor.matmul(
                                    po[m2 * 2 + n][:],
                                    yT[k][:, m2 * 128:(m2 + 1) * 128],
                                    wout[k][:, n * 512:(n + 1) * 512],
                                    start=(k == 0),
                                    stop=(k == KT - 1 and not with_out_bias))
                    if with_out_bias:
                        for m2 in range(2):
                            for n in range(2):
                                nc.tensor.matmul(
                                    po[m2 * 2 + n][:], ones1[:],
                                    bout[:, n * 512:(n + 1) * 512],
                                    start=False, stop=True)
                    for m2 in range(2):
                        for n in range(2):
                            osb = pbp.tile([128, 512], F32,
                                           name=f"osb{b}_{m2}_{n}", tag="osb",
                                           bufs=4)
                            nc.vector.tensor_copy(osb[:], po[m2 * 2 + n][:])
                            nc.sync.dma_start(
                                out_v[m2, b][:, n * 512:(n + 1) * 512], osb[:])

    nc.compile()
    return nc


def _prep_inputs(data, W_qkv, b_qkv, pos_bias_param, W_out, b_out):
    bf = ml_dtypes.bfloat16
    data = np.asarray(data, np.float32)
    W_qkv = np.asarray(W_qkv, np.float32)
    b_qkv = np.asarray(b_qkv, np.float32)
    pos_bias_param = np.asarray(pos_bias_param, np.float32)
    W_out = np.asarray(W_out, np.float32)
    b_out = np.asarray(b_out, np.float32)

    with_qkv_bias = bool(np.any(b_qkv))
    with_out_bias = bool(np.any(b_out))

    wqkv = W_qkv.astype(bf)
    wout = W_out.astype(bf)
    pbT = np.ascontiguousarray(pos_bias_param.T)  # [j, i]

    in_maps = []
    for c in range(N_CORES):
        sl = slice(c * TL, (c + 1) * TL)
        dT = np.ascontiguousarray(
            data[sl].reshape(TOK, D).T).astype(bf)          # [d_in, tok]
        pbT_c = np.ascontiguousarray(pbT[:, sl]).astype(bf)  # [j, i_loc]
        m = {"dataT": dT, "wqkv": wqkv, "pbT": pbT_c, "wout": wout}
        if with_qkv_bias:
            m["bqkv"] = b_qkv.reshape(1, 3 * D).astype(bf)
        if with_out_bias:
            m["bout"] = b_out.reshape(1, D).astype(bf)
        in_maps.append(m)
    return in_maps, with_qkv_bias, with_out_bias


def run(data, W_qkv, b_qkv, pos_bias_param, W_out, b_out, **spmd_kwargs):
    in_maps, wb, ob = _prep_inputs(data, W_qkv, b_qkv, pos_bias_param, W_out, b_out)
    key = (wb, ob)
    if key not in _cache:
        _cache[key] = build(wb, ob)
    nc = _cache[key]
    res = run_bass_kernel_spmd(nc, in_maps, core_ids=list(range(N_CORES)),
                               **spmd_kwargs)
    out = np.concatenate([r["out"] for r in res.results], axis=0)
    return out.reshape(T, B, D), res


def kernel(data, W_qkv, b_qkv, pos_bias_param, W_out, b_out):
    out, _ = run(data, W_qkv, b_qkv, pos_bias_param, W_out, b_out)
    return out
